# revision 3
# baseline (speedup 1.0000x reference)
"""Trainium2 Bass kernel for a 2-layer shared-weight LSTM with residual.

Problem: x:[1024,200,128], W/U:[128,512], b:[512]; two stacked LSTM layers
sharing (W,U,b); layer 2 has a residual connection; seq_len is ignored by the
reference (full T steps).

Sharding: data-parallel over batch: 1024 = 8 cores x 128 rows.

Device layout ("orientation B"): features/gates on SBUF partitions, batch on
the free axis.  Host pre-transposes x to [T, D, B_local] so each timestep tile
is [D=128 partitions, B=128 free] and DMAs straight in.

Fused-unit schedule: unit u (u=0..T) computes layer-2 step u-1 and layer-1
step u together.  For each gate chunk k the PSUM tile holds
    cols 0:128   = z2(u-1) = W_k^T h1(u-1) + U_k^T h2n(u-2) + b_k
    cols 128:256 = z1(u)   = W_k^T x(u)    + U_k^T h1(u-1)  + b_k
so one W-matmul with rhs=[h1|x] (N=256) plus two U-matmuls (N=128 each, split
so the L1 recurrence does not wait on the layer-2 residual) produce both.
Gate activations run on ScalarE with the bias fused (bias is per-partition
because gates live on partitions).  The c/h pointwise ops are split into L1
and L2 halves so the critical recurrence h1 -> z1 -> h1 stays short.
"""

import numpy as np

import concourse.bass as bass
import concourse.tile as tile
from concourse import bacc, mybir
from concourse.bass_utils import run_bass_kernel_spmd

B, T, D = 1024, 200, 128
NCORES = 8
BL = B // NCORES  # 128 batch rows per core

F32 = mybir.dt.float32

# gate order in W/U/b: i, f, g, o  (Keras LSTMCell)
GI, GF, GG, GO = 0, 1, 2, 3


def _build(nc):
    x_d = nc.dram_tensor("x", [T, D, BL], F32, kind="ExternalInput")
    w_d = nc.dram_tensor("w", [D, 4 * D], F32, kind="ExternalInput")
    u_d = nc.dram_tensor("u", [D, 4 * D], F32, kind="ExternalInput")
    b_d = nc.dram_tensor("bias", [D, 4], F32, kind="ExternalInput")
    y_d = nc.dram_tensor("y", [T, D, BL], F32, kind="ExternalOutput")

    SIG = mybir.ActivationFunctionType.Sigmoid
    TANH = mybir.ActivationFunctionType.Tanh

    with tile.TileContext(nc) as tc:
        with (
            tc.tile_pool(name="singles", bufs=1) as singles,
            tc.tile_pool(name="hbuf", bufs=4) as hpool,
            tc.tile_pool(name="psum", bufs=2, space="PSUM") as pspool,
            tc.tile_pool(name="gates", bufs=2) as gpool,
        ):
            w_sb = singles.tile([D, 4 * D], F32)
            u_sb = singles.tile([D, 4 * D], F32)
            b_sb = singles.tile([D, 4], F32)
            nc.sync.dma_start(w_sb[:], w_d[:])
            nc.sync.dma_start(u_sb[:], u_d[:])
            nc.sync.dma_start(b_sb[:], b_d[:])

            # persistent cell state: cols 0:128 = c2, cols 128:256 = c1
            c_both = singles.tile([D, 2 * BL], F32)
            nc.vector.memset(c_both[:], 0.0)

            L2 = slice(0, BL)           # layer-2 half (cols 0:128)
            L1 = slice(BL, 2 * BL)      # layer-1 half (cols 128:256)

            def wk(k):
                return w_sb[:, k * D:(k + 1) * D]

            def uk(k):
                return u_sb[:, k * D:(k + 1) * D]

            def bk(k):
                return b_sb[:, k:k + 1]

            # ring buffer: [h2n(u-2) | h1(u-1) | x(u)]
            bufs = []

            def new_buf():
                t_ = hpool.tile([D, 3 * BL], F32, tag="hbuf", name="hbuf")
                bufs.append(t_)
                return t_

            buf_cur = new_buf()
            nc.sync.dma_start(buf_cur[:, 2 * BL:3 * BL], x_d[0])

            # ---------------- unit 0: layer-1 step 0 only ----------------
            buf_next = new_buf()
            nc.vector.memset(buf_next[:, 0:BL], 0.0)  # h2 state for L2 step 0
            nc.sync.dma_start(buf_next[:, 2 * BL:3 * BL], x_d[1])

            ps = {k: pspool.tile([D, 2 * BL], F32, tag=f"ps{k}", name=f"ps{k}") for k in range(4)}
            for k in (GF, GI, GG, GO):
                # z1(0) = W_k^T x(0) (+ b in ACT); h1(-1)=0 so no U part
                nc.tensor.matmul(ps[k][:, L1], wk(k), buf_cur[:, 2 * BL:3 * BL],
                                 start=True, stop=True)
            gfL1 = gpool.tile([D, BL], F32, tag="gfL1")
            giL1 = gpool.tile([D, BL], F32, tag="giL1")
            ggL1 = gpool.tile([D, BL], F32, tag="ggL1")
            goL1 = gpool.tile([D, BL], F32, tag="goL1")
            tcL1 = gpool.tile([D, BL], F32, tag="tcL1")
            nc.scalar.activation(gfL1[:], ps[GF][:, L1], SIG, bias=bk(GF))
            nc.scalar.activation(giL1[:], ps[GI][:, L1], SIG, bias=bk(GI))
            nc.scalar.activation(ggL1[:], ps[GG][:, L1], TANH, bias=bk(GG))
            nc.scalar.activation(goL1[:], ps[GO][:, L1], SIG, bias=bk(GO))
            # c1(0) = sig(i)*tanh(g)   (c1(-1)=0)
            nc.vector.tensor_mul(c_both[:, L1], giL1[:], ggL1[:])
            nc.scalar.activation(tcL1[:], c_both[:, L1], TANH)
            nc.vector.tensor_mul(buf_next[:, L1], goL1[:], tcL1[:])  # h1(0)

            buf_cur = buf_next

            # ---------------- units 1..T-1: fused L2(u-1) + L1(u) ----------------
            for u in range(1, T):
                buf_next = new_buf()
                if u + 1 < T:
                    nc.sync.dma_start(buf_next[:, 2 * BL:3 * BL], x_d[u + 1])

                ps = {k: pspool.tile([D, 2 * BL], F32, tag=f"ps{k}", name=f"ps{k}")
                      for k in range(4)}
                # W matmuls (need only h1(u-1), x(u)) then L1-side U matmuls
                for k in (GF, GI, GG, GO):
                    nc.tensor.matmul(ps[k][:, 0:2 * BL], wk(k),
                                     buf_cur[:, BL:3 * BL],
                                     start=True, stop=False)
                for k in (GF, GI, GG, GO):
                    nc.tensor.matmul(ps[k][:, L1], uk(k), buf_cur[:, BL:2 * BL],
                                     start=False, stop=True)
                # L2-side U matmuls (need h2n(u-2), i.e. prev unit's residual)
                for k in (GF, GI, GG, GO):
                    nc.tensor.matmul(ps[k][:, L2], uk(k), buf_cur[:, 0:BL],
                                     start=False, stop=True)

                gfL1 = gpool.tile([D, BL], F32, tag="gfL1")
                giL1 = gpool.tile([D, BL], F32, tag="giL1")
                ggL1 = gpool.tile([D, BL], F32, tag="ggL1")
                goL1 = gpool.tile([D, BL], F32, tag="goL1")
                tcL1 = gpool.tile([D, BL], F32, tag="tcL1")
                gfL2 = gpool.tile([D, BL], F32, tag="gfL2")
                giL2 = gpool.tile([D, BL], F32, tag="giL2")
                ggL2 = gpool.tile([D, BL], F32, tag="ggL2")
                goL2 = gpool.tile([D, BL], F32, tag="goL2")
                tcL2 = gpool.tile([D, BL], F32, tag="tcL2")
                m1a = gpool.tile([D, BL], F32, tag="m1a")
                m2a = gpool.tile([D, BL], F32, tag="m2a")
                m1b = gpool.tile([D, BL], F32, tag="m1b")
                m2b = gpool.tile([D, BL], F32, tag="m2b")
                h2raw = gpool.tile([D, BL], F32, tag="h2raw")

                # ---- L1 chain (critical): gates -> c1 -> tanh -> h1(u)
                nc.scalar.activation(gfL1[:], ps[GF][:, L1], SIG, bias=bk(GF))
                nc.scalar.activation(giL1[:], ps[GI][:, L1], SIG, bias=bk(GI))
                nc.scalar.activation(ggL1[:], ps[GG][:, L1], TANH, bias=bk(GG))
                nc.vector.tensor_mul(m1a[:], gfL1[:], c_both[:, L1])
                nc.gpsimd.tensor_mul(m2a[:], giL1[:], ggL1[:])
                nc.vector.tensor_add(c_both[:, L1], m1a[:], m2a[:])
                nc.scalar.activation(goL1[:], ps[GO][:, L1], SIG, bias=bk(GO))
                nc.scalar.activation(tcL1[:], c_both[:, L1], TANH)
                nc.vector.tensor_mul(buf_next[:, L1], goL1[:], tcL1[:])  # h1(u)

                # ---- L2 half: gates -> c2 -> tanh -> h2raw -> +h1(u-1)
                nc.scalar.activation(gfL2[:], ps[GF][:, L2], SIG, bias=bk(GF))
                nc.scalar.activation(giL2[:], ps[GI][:, L2], SIG, bias=bk(GI))
                nc.scalar.activation(ggL2[:], ps[GG][:, L2], TANH, bias=bk(GG))
                nc.vector.tensor_mul(m1b[:], gfL2[:], c_both[:, L2])
                nc.gpsimd.tensor_mul(m2b[:], giL2[:], ggL2[:])
                nc.vector.tensor_add(c_both[:, L2], m1b[:], m2b[:])
                nc.scalar.activation(goL2[:], ps[GO][:, L2], SIG, bias=bk(GO))
                nc.scalar.activation(tcL2[:], c_both[:, L2], TANH)
                nc.gpsimd.tensor_mul(h2raw[:], goL2[:], tcL2[:])
                # h2n(u-1) = h2raw + h1(u-1): residual, also the y output
                nc.vector.tensor_add(buf_next[:, 0:BL], h2raw[:],
                                     buf_cur[:, BL:2 * BL])
                nc.sync.dma_start(y_d[u - 1], buf_next[:, 0:BL])

                buf_cur = buf_next

            # ---------------- unit T: layer-2 step T-1 only ----------------
            buf_next = new_buf()
            ps = {k: pspool.tile([D, 2 * BL], F32, tag=f"ps{k}", name=f"ps{k}") for k in range(4)}
            for k in (GF, GI, GG, GO):
                nc.tensor.matmul(ps[k][:, L2], wk(k), buf_cur[:, BL:2 * BL],
                                 start=True, stop=False)
                nc.tensor.matmul(ps[k][:, L2], uk(k), buf_cur[:, 0:BL],
                                 start=False, stop=True)
            gfL2 = gpool.tile([D, BL], F32, tag="gfL2")
            giL2 = gpool.tile([D, BL], F32, tag="giL2")
            ggL2 = gpool.tile([D, BL], F32, tag="ggL2")
            goL2 = gpool.tile([D, BL], F32, tag="goL2")
            tcL2 = gpool.tile([D, BL], F32, tag="tcL2")
            m1b = gpool.tile([D, BL], F32, tag="m1b")
            m2b = gpool.tile([D, BL], F32, tag="m2b")
            h2raw = gpool.tile([D, BL], F32, tag="h2raw")
            nc.scalar.activation(gfL2[:], ps[GF][:, L2], SIG, bias=bk(GF))
            nc.scalar.activation(giL2[:], ps[GI][:, L2], SIG, bias=bk(GI))
            nc.scalar.activation(ggL2[:], ps[GG][:, L2], TANH, bias=bk(GG))
            nc.scalar.activation(goL2[:], ps[GO][:, L2], SIG, bias=bk(GO))
            nc.vector.tensor_mul(m1b[:], gfL2[:], c_both[:, L2])
            nc.gpsimd.tensor_mul(m2b[:], giL2[:], ggL2[:])
            nc.vector.tensor_add(c_both[:, L2], m1b[:], m2b[:])
            nc.scalar.activation(tcL2[:], c_both[:, L2], TANH)
            nc.gpsimd.tensor_mul(h2raw[:], goL2[:], tcL2[:])
            nc.vector.tensor_add(buf_next[:, 0:BL], h2raw[:],
                                 buf_cur[:, BL:2 * BL])
            nc.sync.dma_start(y_d[T - 1], buf_next[:, 0:BL])

    nc.finalize()
    return nc


_CACHED = {}


def _get_nc():
    if "nc" not in _CACHED:
        nc = bacc.Bacc("TRN2", target_bir_lowering=False, debug=False,
                       num_devices=NCORES)
        _CACHED["nc"] = _build(nc)
    return _CACHED["nc"]


def kernel(x, W, U, b, seq_len):
    assert x.shape == (B, T, D)
    nc = _get_nc()

    Wc = np.ascontiguousarray(W, dtype=np.float32)
    Uc = np.ascontiguousarray(U, dtype=np.float32)
    bc = np.ascontiguousarray(
        np.asarray(b, dtype=np.float32).reshape(4, D).T)  # [D, 4]

    in_maps = []
    for c in range(NCORES):
        xc = np.ascontiguousarray(
            np.asarray(x[c * BL:(c + 1) * BL], dtype=np.float32)
            .transpose(1, 2, 0))  # [T, D, BL]
        in_maps.append({"x": xc, "w": Wc, "u": Uc, "bias": bc})

    res = run_bass_kernel_spmd(nc, in_maps, core_ids=list(range(NCORES)))

    y = np.empty((B, T, D), dtype=np.float32)
    for c in range(NCORES):
        # y_T [T, D, BL] -> [BL, T, D]
        y[c * BL:(c + 1) * BL] = res.results[c]["y"].transpose(2, 0, 1)
    return y


# revision 8
# speedup vs baseline: 1.7633x; 1.7633x over previous
"""Trainium2 Bass kernel for a 2-layer shared-weight LSTM with residual.

Problem: x:[1024,200,128], W/U:[128,512], b:[512]; two stacked LSTM layers
sharing (W,U,b); layer 2 has a residual connection; seq_len is ignored by the
reference (full T steps).

Sharding: data-parallel over batch: 1024 = 8 cores x 128 rows.

Device layout ("orientation B"): features/gates on SBUF partitions, batch on
the free axis.  Host pre-transposes x to [T, D, B_local] (bf16) so each
timestep tile is [D=128 partitions, B=128 free] and DMAs straight in.

Fused-unit schedule: unit u (u=0..T) computes layer-2 step u-1 and layer-1
step u together.  For each gate chunk k the PSUM tile holds
    cols 0:128   = z2(u-1) = W_k h1(u-1) + U_k h2raw(u-2) + U_k h1(u-2) + b_k
    cols 128:256 = z1(u)   = W_k x(u)    + U_k h1(u-1)                  + b_k
The layer-2 recurrent input h2n = h2raw + h1 is split across two matmuls so
the residual add is off the recurrence cycle entirely (it only feeds the y
output DMA, on GpSimd).  Matmuls whose inputs are a unit old (U_k h1(u-2),
W_k x(u)) are issued early so only three N=128 matmuls sit between h-ready
and the first gate activation.  Gate activations are merged [128,256]
ScalarE ops (bias fused; per-partition because gates live on partitions);
the c/h pointwise tail is split into L1/L2 halves to shorten the serial
recurrence.  Matmuls run in bf16 (fp32 runs 2-pass LOW_HIGH at half speed);
the c state stays fp32.
"""

import numpy as np
import ml_dtypes

import concourse.bass as bass
import concourse.tile as tile
from concourse import bacc, mybir
from concourse.bass_utils import run_bass_kernel_spmd

B, T, D = 1024, 200, 128
NCORES = 8
BL = B // NCORES  # 128 batch rows per core

F32 = mybir.dt.float32
import os
BF16 = mybir.dt.float32 if os.environ.get("K_FP32") else mybir.dt.bfloat16

# gate order in W/U/b: i, f, g, o  (Keras LSTMCell)
GI, GF, GG, GO = 0, 1, 2, 3
CHUNKS = (GF, GI, GG, GO)  # f first: the c-path needs sig(f) earliest


def _build(nc):
    x_d = nc.dram_tensor("x", [T, D, BL], BF16, kind="ExternalInput")
    w_d = nc.dram_tensor("w", [D, 4 * D], BF16, kind="ExternalInput")
    u_d = nc.dram_tensor("u", [D, 4 * D], BF16, kind="ExternalInput")
    b_d = nc.dram_tensor("bias", [D, 4], F32, kind="ExternalInput")
    y_d = nc.dram_tensor("y", [T, D, BL], BF16, kind="ExternalOutput")

    SIG = mybir.ActivationFunctionType.Sigmoid
    TANH = mybir.ActivationFunctionType.Tanh

    L2 = slice(0, BL)           # layer-2 half (cols 0:128)
    L1 = slice(BL, 2 * BL)      # layer-1 half (cols 128:256)

    with tile.TileContext(nc) as tc:
        with (
            tc.tile_pool(name="singles", bufs=1) as singles,
            tc.tile_pool(name="hbuf", bufs=5) as hpool,
            tc.tile_pool(name="psum", bufs=2, space="PSUM") as pspool,
            tc.tile_pool(name="gates", bufs=2) as gpool,
            tc.tile_pool(name="yst", bufs=3) as ypool,
        ):
            w_sb = singles.tile([D, 4 * D], BF16)
            u_sb = singles.tile([D, 4 * D], BF16)
            b_sb = singles.tile([D, 4], F32)
            nc.sync.dma_start(w_sb[:], w_d[:])
            nc.sync.dma_start(u_sb[:], u_d[:])
            nc.sync.dma_start(b_sb[:], b_d[:])

            # persistent cell state: cols 0:128 = c2, cols 128:256 = c1
            c_both = singles.tile([D, 2 * BL], F32)
            nc.vector.memset(c_both[:], 0.0)

            def wk(k):
                return w_sb[:, k * D:(k + 1) * D]

            def uk(k):
                return u_sb[:, k * D:(k + 1) * D]

            def bk(k):
                return b_sb[:, k:k + 1]

            # ring: hb[u] = [h2raw(u-1) | h1(u) | x(u+1)]   (bf16)
            def new_hb():
                return hpool.tile([D, 3 * BL], BF16, tag="hbuf", name="hbuf")

            def new_ps():
                return {k: pspool.tile([D, 2 * BL], F32, tag=f"ps{k}",
                                       name=f"ps{k}") for k in range(4)}

            def new_gates(names):
                return {n: gpool.tile([D, 2 * BL], BF16, tag=n, name=n)
                        for n in names}

            # hb_pre carries x(0) for unit 0
            hb_pre = new_hb()
            nc.sync.dma_start(hb_pre[:, 2 * BL:3 * BL], x_d[0])

            # ---------------- unit 0: layer-1 step 0 only ----------------
            # z1(0) = W x(0) + b ;  c1(0) = sig(i)*tanh(g) ; h1(0)=sig(o)*tanh(c1)
            hb0 = new_hb()
            nc.sync.dma_start(hb0[:, 2 * BL:3 * BL], x_d[1])
            ps = new_ps()
            for k in CHUNKS:
                nc.tensor.matmul(ps[k][:, L1], wk(k),
                                 hb_pre[:, 2 * BL:3 * BL],
                                 start=True, stop=True)
            g = new_gates(["gf", "gi", "gg", "go", "tc1", "tc2"])
            nc.scalar.activation(g["gi"][:, L1], ps[GI][:, L1], SIG, bias=bk(GI))
            nc.scalar.activation(g["gg"][:, L1], ps[GG][:, L1], TANH, bias=bk(GG))
            nc.scalar.activation(g["go"][:, L1], ps[GO][:, L1], SIG, bias=bk(GO))
            nc.vector.tensor_mul(c_both[:, L1], g["gi"][:, L1], g["gg"][:, L1])
            nc.scalar.activation(g["tc1"][:, L1], c_both[:, L1], TANH)
            nc.vector.tensor_mul(hb0[:, L1], g["go"][:, L1], g["tc1"][:, L1])

            hb = {-1: hb_pre, 0: hb0}

            # -------- units 1..T-1: fused L2(u-1) + L1(u) --------
            for u in range(1, T):
                hb_u = new_hb()
                hb[u] = hb_u
                if u + 1 < T:
                    nc.sync.dma_start(hb_u[:, 2 * BL:3 * BL], x_d[u + 1])

                ps = new_ps()
                # Per chunk: one full-width W matmul opens the tile (single
                # start=True per bank), then U accumulates: L1 += U h1(u-1);
                # L2 += U h1(u-2) + U h2raw(u-2)  (= U h2n(u-2), residual
                # decomposed so it never waits on the residual add).
                for k in CHUNKS:
                    nc.tensor.matmul(ps[k][:, 0:2 * BL], wk(k),
                                     hb[u - 1][:, BL:3 * BL],
                                     start=True, stop=False)
                    nc.tensor.matmul(ps[k][:, L1], uk(k),
                                     hb[u - 1][:, BL:2 * BL],
                                     start=False, stop=True)
                    if u >= 2:
                        nc.tensor.matmul(ps[k][:, L2], uk(k),
                                         hb[u - 2][:, BL:2 * BL],
                                         start=False, stop=False)
                        nc.tensor.matmul(ps[k][:, L2], uk(k),
                                         hb[u - 1][:, 0:BL],
                                         start=False, stop=True)

                g = new_gates(["gf", "gi", "gg", "go", "tc1", "tc2"])
                m = {n: gpool.tile([D, 2 * BL], F32, tag=n, name=n)
                     for n in ("m1a", "m1b")}
                m.update({n: gpool.tile([D, 2 * BL], BF16, tag=n, name=n)
                          for n in ("m2a", "m2b")})

                # merged gate activations [128,256] (both layer-halves)
                nc.scalar.activation(g["gf"][:], ps[GF][:], SIG, bias=bk(GF))
                nc.scalar.activation(g["gi"][:], ps[GI][:], SIG, bias=bk(GI))
                nc.scalar.activation(g["gg"][:], ps[GG][:], TANH, bias=bk(GG))
                nc.scalar.activation(g["go"][:], ps[GO][:], SIG, bias=bk(GO))

                # L1 tail (critical recurrence): c1' -> tanh -> h1(u)
                nc.vector.tensor_mul(m["m1a"][:, L1], g["gf"][:, L1],
                                     c_both[:, L1])
                nc.vector.tensor_mul(m["m2a"][:, L1], g["gi"][:, L1],
                                     g["gg"][:, L1])
                nc.vector.tensor_add(c_both[:, L1], m["m1a"][:, L1],
                                     m["m2a"][:, L1])
                nc.scalar.activation(g["tc1"][:, L1], c_both[:, L1], TANH)
                # L2 tail: c2' -> tanh -> h2raw(u-1)
                nc.vector.tensor_mul(m["m1b"][:, L2], g["gf"][:, L2],
                                     c_both[:, L2])
                nc.vector.tensor_mul(m["m2b"][:, L2], g["gi"][:, L2],
                                     g["gg"][:, L2])
                nc.vector.tensor_mul(hb_u[:, L1], g["go"][:, L1],
                                     g["tc1"][:, L1])          # h1(u)
                nc.vector.tensor_add(c_both[:, L2], m["m1b"][:, L2],
                                     m["m2b"][:, L2])
                nc.scalar.activation(g["tc2"][:, L2], c_both[:, L2], TANH)
                nc.vector.tensor_mul(hb_u[:, L2], g["go"][:, L2],
                                     g["tc2"][:, L2])          # h2raw(u-1)

                # residual -> y(u-1): off the recurrence, on GpSimd
                yt = ypool.tile([D, BL], BF16, tag="yst", name="yst")
                nc.gpsimd.tensor_add(yt[:], hb_u[:, 0:BL],
                                     hb[u - 1][:, BL:2 * BL])
                nc.sync.dma_start(y_d[u - 1], yt[:])

                hb.pop(u - 3, None)

            # ---------------- unit T: layer-2 step T-1 only ----------------
            u = T
            ps = new_ps()
            for k in CHUNKS:
                nc.tensor.matmul(ps[k][:, L2], uk(k), hb[u - 2][:, BL:2 * BL],
                                 start=True, stop=False)
                nc.tensor.matmul(ps[k][:, L2], wk(k), hb[u - 1][:, BL:2 * BL],
                                 start=False, stop=False)
                nc.tensor.matmul(ps[k][:, L2], uk(k), hb[u - 1][:, 0:BL],
                                 start=False, stop=True)
            g = new_gates(["gf", "gi", "gg", "go", "tc2"])
            m = {"m1b": gpool.tile([D, 2 * BL], F32, tag="m1b", name="m1b"),
                 "m2b": gpool.tile([D, 2 * BL], BF16, tag="m2b", name="m2b")}
            nc.scalar.activation(g["gf"][:, L2], ps[GF][:, L2], SIG, bias=bk(GF))
            nc.scalar.activation(g["gi"][:, L2], ps[GI][:, L2], SIG, bias=bk(GI))
            nc.scalar.activation(g["gg"][:, L2], ps[GG][:, L2], TANH, bias=bk(GG))
            nc.scalar.activation(g["go"][:, L2], ps[GO][:, L2], SIG, bias=bk(GO))
            nc.vector.tensor_mul(m["m1b"][:, L2], g["gf"][:, L2], c_both[:, L2])
            nc.vector.tensor_mul(m["m2b"][:, L2], g["gi"][:, L2], g["gg"][:, L2])
            nc.vector.tensor_add(c_both[:, L2], m["m1b"][:, L2], m["m2b"][:, L2])
            nc.scalar.activation(g["tc2"][:, L2], c_both[:, L2], TANH)
            hraw = ypool.tile([D, BL], BF16, tag="yst", name="hraw")
            nc.vector.tensor_mul(hraw[:], g["go"][:, L2], g["tc2"][:, L2])
            yt = ypool.tile([D, BL], BF16, tag="yst", name="yst")
            nc.gpsimd.tensor_add(yt[:], hraw[:], hb[u - 1][:, BL:2 * BL])
            nc.sync.dma_start(y_d[T - 1], yt[:])

    nc.finalize()
    return nc


_CACHED = {}


def _get_nc():
    if "nc" not in _CACHED:
        nc = bacc.Bacc("TRN2", target_bir_lowering=False, debug=False,
                       num_devices=NCORES)
        _CACHED["nc"] = _build(nc)
    return _CACHED["nc"]


def kernel(x, W, U, b, seq_len):
    assert x.shape == (B, T, D)
    nc = _get_nc()

    import os
    bf = np.float32 if os.environ.get("K_FP32") else ml_dtypes.bfloat16
    Wc = np.ascontiguousarray(np.asarray(W, dtype=np.float32).astype(bf))
    Uc = np.ascontiguousarray(np.asarray(U, dtype=np.float32).astype(bf))
    bc = np.ascontiguousarray(
        np.asarray(b, dtype=np.float32).reshape(4, D).T)  # [D, 4]

    in_maps = []
    for c in range(NCORES):
        xc = np.ascontiguousarray(
            np.asarray(x[c * BL:(c + 1) * BL], dtype=np.float32)
            .transpose(1, 2, 0).astype(bf))  # [T, D, BL] bf16
        in_maps.append({"x": xc, "w": Wc, "u": Uc, "bias": bc})

    res = run_bass_kernel_spmd(nc, in_maps, core_ids=list(range(NCORES)))

    y = np.empty((B, T, D), dtype=np.float32)
    for c in range(NCORES):
        # y_T [T, D, BL] bf16 -> [BL, T, D] fp32
        y[c * BL:(c + 1) * BL] = (
            res.results[c]["y"].astype(np.float32).transpose(2, 0, 1))
    return y
